# revision 3
# baseline (speedup 1.0000x reference)
"""Trainium2 Bass kernel for CustomSelfAttentionWithBias (B=2, T=2048, C=1024, H=16).

Computes y = proj(softmax(mask(QK^T/sqrt(hd) + emphasis_col0)) @ V) where
qkv = x @ W_attn, with a causal bool mask and +1.0 emphasis on score column 0.

Sharding: 8 cores; core c handles batch b = c//4 and heads 4*(c%4) .. +4
(data parallel on B, tensor parallel on heads; c_proj row-sharded so each
core emits a partial y[b] that the host sums).

Dataflow per core (everything bf16 into the PE, fp32 PSUM):
  - host pre-transposes x[b] -> xT [C, T] and pre-slices/casts weights (bf16),
    pre-scales Wq by 1/sqrt(hd).
  - Q^T,K^T [64,T] per head and V [T,64] per head come straight out of
    matmuls against xT (no on-chip transposes anywhere).
  - scores are computed transposed: S^T[k_chunk 128, q 512] = K^T.T @ Q^T;
    the two heads of a pair run as one PE pass via disjoint row groups.
  - exp on ScalarE (PSUM -> SBUF bf16), causal masking by multiplying the
    diagonal chunks with a precomputed 0/1 slab.
  - PV with lhsT = [V | ones]: one accumulation produces O^T[64, q] AND the
    softmax denominator row; normalization happens in the PSUM->SBUF copy
    (multiply by DMA-broadcast reciprocal of the denominator).
  - proj: y[t 128, c 512] accumulated over the 2 head-pair chunks, copied to
    fp16 and DMA'd out; host sums the 4 partials per batch in fp32.

Scheduling: fully software-pipelined. x arrives in 4 token-block DMAs
(weights spread over other engine queues) so the PE starts ~3us in; QKV
generation for token block nb+1 and the projection of query block qb-1 are
woven between the attention chunks of query block qb, so the PE never
waits on the Scalar exp and no phase runs serially.
"""

import numpy as np
import ml_dtypes

B, T, C = 2, 2048, 1024
H, HD = 16, 64
NH = 4            # heads per core
N_CORES = 8
QB = 512          # query block (columns of S^T per matmul)
KC = 128          # key chunk (partition dim of S^T)
N_QB = T // QB    # 4
N_KC = T // KC    # 16
CCH = C // 128    # 8 contraction chunks for the projections
EMPHASIS = 1.0

_COMPILED = {}


def _build(causal: bool = True):
    import concourse.bass as bass
    import concourse.tile as tile
    import concourse.mybir as mybir
    from concourse import bacc

    f32 = mybir.dt.float32
    f16 = mybir.dt.float16
    bf16 = mybir.dt.bfloat16
    EXP = mybir.ActivationFunctionType.Exp

    nc = bacc.Bacc("TRN2", target_bir_lowering=False, debug=False)

    xT = nc.dram_tensor("xT", [C, T], bf16, kind="ExternalInput").ap()
    wq = nc.dram_tensor("wq", [C, NH * HD], bf16, kind="ExternalInput").ap()
    wk = nc.dram_tensor("wk", [C, NH * HD], bf16, kind="ExternalInput").ap()
    wv = nc.dram_tensor("wv", [C, NH * HD], bf16, kind="ExternalInput").ap()
    wp = nc.dram_tensor("wp", [NH * HD, C], bf16, kind="ExternalInput").ap()
    mk = nc.dram_tensor("mk", [128, 896], bf16, kind="ExternalInput").ap()
    y = nc.dram_tensor("y", [T, C], f16, kind="ExternalOutput").ap()

    with tile.TileContext(nc) as tc:
        _body(nc, tc, bass, mybir, xT, wq, wk, wv, wp, mk, y, causal,
              f32, f16, bf16, EXP)
    nc.compile()
    return nc


def _body(nc, tc, bass, mybir, xT, wq, wk, wv, wp, mk, y, causal,
          f32, f16, bf16, EXP):
    from contextlib import ExitStack
    import math

    ctx = ExitStack()
    singles = ctx.enter_context(tc.tile_pool(name="singles", bufs=1))
    # PSUM: st 2x2 banks + po 2x1 + aux 2x1 = 8 banks exactly
    ps = ctx.enter_context(tc.tile_pool(name="ps", bufs=2, space="PSUM"))
    pt_pool = ctx.enter_context(tc.tile_pool(name="pt_pool", bufs=4))
    nrm_pool = ctx.enter_context(tc.tile_pool(name="nrm_pool", bufs=3))
    y_pool = ctx.enter_context(tc.tile_pool(name="y_pool", bufs=3))

    # ---- input DMAs: x split by token block, weights on other queues ----
    xT_r = xT.rearrange("(c p) t -> p c t", p=128)
    xn = []
    for nb in range(N_QB):
        t = singles.tile([128, CCH, QB], bf16, name=f"xn{nb}")
        nc.sync.dma_start(out=t, in_=xT_r[:, :, nb * QB:(nb + 1) * QB])
        xn.append(t)
    wq_sb = singles.tile([128, CCH, NH * HD], bf16, name="wq_sb")
    nc.gpsimd.dma_start(out=wq_sb, in_=wq.rearrange("(c p) n -> p c n", p=128))
    wk_sb = singles.tile([128, CCH, NH * HD], bf16, name="wk_sb")
    nc.scalar.dma_start(out=wk_sb, in_=wk.rearrange("(c p) n -> p c n", p=128))
    wv_sb = singles.tile([128, CCH, NH * HD], bf16, name="wv_sb")
    nc.scalar.dma_start(out=wv_sb, in_=wv.rearrange("(c p) n -> p c n", p=128))
    wp_sb = singles.tile([128, 2, C], bf16, name="wp_sb")
    nc.gpsimd.dma_start(out=wp_sb, in_=wp.rearrange("(j p) n -> p j n", p=128))
    mk_sb = singles.tile([128, 896], bf16, name="mk_sb")
    nc.scalar.dma_start(out=mk_sb, in_=mk)

    # ---- per-block resident tensors -------------------------------------
    qt_sb = [[singles.tile([128, QB], bf16, name=f"qt{pr}_{nb}")
              for nb in range(N_QB)] for pr in range(2)]
    kt_sb = [[singles.tile([128, QB], bf16, name=f"kt{pr}_{nb}")
              for nb in range(N_QB)] for pr in range(2)]
    ot_sb = [[singles.tile([128, QB], bf16, name=f"ot{pr}_{qb}")
              for qb in range(N_QB)] for pr in range(2)]
    v_t = [singles.tile([128, NH, HD + 1], bf16, name=f"v{kc}")
           for kc in range(N_KC)]

    # ---- pipeline unit emitters -----------------------------------------
    def unit_qt(pr, nb, dst, w_sb):
        def emit():
            pg = ps.tile([128, QB], f32, tag="aux", name="pg")
            for cc in range(CCH):
                nc.tensor.matmul(
                    pg, w_sb[:, cc, pr * 128:(pr + 1) * 128], xn[nb][:, cc, :],
                    start=(cc == 0), stop=(cc == CCH - 1))
            nc.vector.tensor_copy(dst, pg)
        return emit

    def unit_v(kc):
        def emit():
            nb, j = kc // 4, kc % 4
            pg = ps.tile([128, NH, HD], f32, tag="aux", name="pgv")
            for cc in range(CCH):
                nc.tensor.matmul(
                    pg, xn[nb][:, cc, j * 128:(j + 1) * 128], wv_sb[:, cc, :],
                    start=(cc == 0), stop=(cc == CCH - 1))
            nc.gpsimd.memset(v_t[kc][:, :, HD:HD + 1], 1.0)
            nc.vector.tensor_copy(v_t[kc][:, :, 0:HD], pg)
            if kc == 0:
                # fold the column-0 emphasis into V|ones row for key 0
                nc.scalar.mul(v_t[0][0:1, :, :], v_t[0][0:1, :, :],
                              float(math.exp(EMPHASIS)))
        return emit

    def unit_proj(qb, tci):
        def emit():
            tg = 4 * qb + tci
            ysb = y_pool.tile([128, C], f16, tag="ysb", name="ysb")
            for ch in range(2):
                py = ps.tile([128, QB], f32, tag="aux", name="py")
                for pr2 in range(2):
                    nc.tensor.matmul(
                        py, ot_sb[pr2][qb][:, tci * 128:(tci + 1) * 128],
                        wp_sb[:, pr2, ch * QB:(ch + 1) * QB],
                        start=(pr2 == 0), stop=(pr2 == 1))
                nc.vector.tensor_copy(ysb[:, ch * QB:(ch + 1) * QB], py)
            nc.sync.dma_start(out=y[tg * 128:(tg + 1) * 128, :], in_=ysb)
        return emit

    # ---- attention helpers ----------------------------------------------
    def norm(h, qb, po):
        # recip(den) broadcast, fused into the O^T PSUM->SBUF copy
        pr, s = h // 2, h % 2
        den = nrm_pool.tile([HD + 1, QB], f32, tag="den", name="den")
        nc.vector.tensor_copy(den[HD:HD + 1, :], po[HD:HD + 1, :])
        bde = nrm_pool.tile([HD, QB], f32, tag="bde", name="bde")
        nc.gpsimd.dma_start(
            out=bde,
            in_=den[HD:HD + 1, :].unsqueeze(1).broadcast_to([1, HD, QB]))
        rec = nrm_pool.tile([HD, QB], f32, tag="rec", name="rec")
        nc.vector.reciprocal_approx_fast(out=rec, in_=bde)
        if s == 0:
            nc.vector.tensor_mul(
                ot_sb[pr][qb][0:HD, :], po[0:HD, :], rec)
        else:
            osh = nrm_pool.tile([HD, QB], bf16, tag="osh", name="osh")
            nc.vector.tensor_mul(osh, po[0:HD, :], rec)
            nc.gpsimd.dma_start(out=ot_sb[pr][qb][HD:128, :], in_=osh)

    pending = []

    def emit_pv(rec_):
        pr, qb, kc, pt, po0, po1, nk = rec_
        nc.tensor.matmul(po0, v_t[kc][:, 2 * pr, :], pt[:, 0, :],
                         start=(kc == 0), stop=(kc == nk - 1))
        nc.tensor.matmul(po1, v_t[kc][:, 2 * pr + 1, :], pt[:, 1, :],
                         start=(kc == 0), stop=(kc == nk - 1))
        if kc == nk - 1:
            # s=1 (with its extra shift DMA) first so the block's last norm
            # chain, which gates the trailing projection, is the short one
            norm(2 * pr + 1, qb, po1)
            norm(2 * pr, qb, po0)

    # ---- prelude: first Q^T/K^T block (everything else is woven in) ----
    unit_qt(0, 0, qt_sb[0][0], wq_sb)()
    unit_qt(0, 0, kt_sb[0][0], wk_sb)()
    if not causal:
        # dense fallback: no phase structure to hide gen behind; do it all up
        # front and only weave the projections.
        unit_qt(1, 0, qt_sb[1][0], wq_sb)()
        unit_qt(1, 0, kt_sb[1][0], wk_sb)()
        for nb in range(1, N_QB):
            for pr in range(2):
                unit_qt(pr, nb, qt_sb[pr][nb], wq_sb)()
                unit_qt(pr, nb, kt_sb[pr][nb], wk_sb)()
        for kc in range(N_KC):
            unit_v(kc)()

    # ---- main software-pipelined loop -----------------------------------
    for qb in range(N_QB):
        nk = 4 * (qb + 1) if causal else N_KC
        F = []
        if causal:
            if qb == 0:
                F += [unit_v(0), unit_v(1),
                      unit_qt(1, 0, qt_sb[1][0], wq_sb),
                      unit_qt(1, 0, kt_sb[1][0], wk_sb),
                      unit_v(2), unit_v(3)]
            nb2 = qb + 1
            if nb2 < N_QB:
                for pr in range(2):
                    F.append(unit_qt(pr, nb2, qt_sb[pr][nb2], wq_sb))
                    F.append(unit_qt(pr, nb2, kt_sb[pr][nb2], wk_sb))
        if qb >= 1:
            F += [unit_proj(qb - 1, tci) for tci in range(4)]
        if causal and qb + 1 < N_QB:
            F += [unit_v(kc) for kc in range(4 * (qb + 1), 4 * (qb + 1) + 4)]

        nch = 2 * nk
        ndone = 0
        ci = 0
        for pr in range(2):
            po0 = ps.tile([HD + 1, QB], f32, tag="po", name="po0")
            po1 = ps.tile([HD + 1, QB], f32, tag="po", name="po1")
            for kc in range(nk):
                st = ps.tile([128, 2, QB], f32, tag="st", name="st")
                for s in range(2):
                    r0, r1 = s * HD, (s + 1) * HD
                    nc.tensor.matmul(
                        st[:, s, :],
                        kt_sb[pr][kc // 4][r0:r1, (kc % 4) * 128:(kc % 4 + 1) * 128],
                        qt_sb[pr][qb][r0:r1, :],
                        start=True, stop=True)
                pt = pt_pool.tile([128, 2, QB], bf16, tag="pt", name="pt")
                nc.scalar.activation(out=pt, in_=st, func=EXP)
                r = kc - 4 * qb
                if causal and r >= 0:
                    m0 = 384 - 128 * r
                    for s in range(2):
                        nc.vector.tensor_mul(
                            pt[:, s, :], pt[:, s, :], mk_sb[:, m0:m0 + QB])
                ci += 1
                want = (ci * len(F) + nch - 1) // nch if F else 0
                while ndone < want:
                    F[ndone]()
                    ndone += 1
                while len(pending) >= 2:
                    emit_pv(pending.pop(0))
                pending.append((pr, qb, kc, pt, po0, po1, nk))
        # flush so this block's norms precede next phase's projections
        while pending:
            emit_pv(pending.pop(0))
        while ndone < len(F):
            F[ndone]()
            ndone += 1

    for tci in range(4):
        unit_proj(N_QB - 1, tci)()

    ctx.close()


def _prep_inputs(x, W_attn, W_proj, attn_mask):
    """Host-side shard + layout prep. Returns (in_maps, causal)."""
    bf = ml_dtypes.bfloat16
    causal = bool(np.array_equal(
        np.asarray(attn_mask),
        np.tril(np.ones((T, T), dtype=bool))))

    x = np.asarray(x, dtype=np.float32)
    Wa = np.asarray(W_attn, dtype=np.float32)
    Wp = np.asarray(W_proj, dtype=np.float32)

    scale = 1.0 / np.sqrt(np.float32(HD))
    xT_b = [np.ascontiguousarray(x[b].T).astype(bf) for b in range(B)]

    # causal diagonal-mask slab: mk[i, m] = 1.0 if i <= m - 384 else 0
    i = np.arange(128)[:, None]
    m = np.arange(896)[None, :]
    mk = (i <= (m - 384)).astype(bf)

    in_maps = []
    for core in range(N_CORES):
        b, h0 = core // 4, (core % 4) * NH
        hsl = slice(h0 * HD, (h0 + NH) * HD)
        wq_c = np.ascontiguousarray(Wa[:, hsl] * scale).astype(bf)
        wk_c = np.ascontiguousarray(Wa[:, C + h0 * HD: C + (h0 + NH) * HD]).astype(bf)
        wv_c = np.ascontiguousarray(Wa[:, 2 * C + h0 * HD: 2 * C + (h0 + NH) * HD]).astype(bf)
        wp_c = np.ascontiguousarray(Wp[hsl, :]).astype(bf)
        in_maps.append({
            "xT": xT_b[b], "wq": wq_c, "wk": wk_c, "wv": wv_c,
            "wp": wp_c, "mk": mk,
        })
    return in_maps, causal


def kernel(x, W_attn, W_proj, attn_mask, _trace=False):
    from concourse import bass_utils

    in_maps, causal = _prep_inputs(x, W_attn, W_proj, attn_mask)
    key = ("causal" if causal else "dense")
    if key not in _COMPILED:
        _COMPILED[key] = _build(causal)
    nc = _COMPILED[key]

    res = bass_utils.run_bass_kernel_spmd(
        nc, in_maps, core_ids=list(range(N_CORES)), trace=_trace)

    y = np.zeros((B, T, C), dtype=np.float32)
    for core in range(N_CORES):
        y[core // 4] += res.results[core]["y"].astype(np.float32)
    if _trace:
        kernel._last_results = res
    return y
